# revision 2
# baseline (speedup 1.0000x reference)
"""GIN message-passing network on 8 Trainium2 NeuronCores.

Strategy
--------
The edge list is converted (on host, O(E)) to a dense transposed adjacency
matrix A_T[src, dst].  segment_sum(h[src], dst) == A @ h, and since the GIN
update is ((1+eps)h + A@h) @ W = (1+eps)(h@W) + A@(h@W), the N x N adjacency
contraction runs at hidden width H=512 instead of 4096.  BatchNorm (eval
mode) + bias are folded on host into per-feature affine scale/shift applied
with fused scalar-engine relu-activations.

Sharding: nodes are split 8 ways (512 nodes per core).  Activations are kept
feature-major ("transposed", [H, nodes]) on each core so that per-feature
affines land on the partition axis and no on-device transposes are needed:

  Z_c   = matmul(lhsT=hT_c,  rhs=Wa)     -> [nodes_c, H] (node-major)
  AllGather(Z_c)                         -> Zfull [N, H]
  XT_c  = matmul(lhsT=Zfull, rhs=AT_c)   -> [H, nodes_c]
        + (1+eps) * Z_c^T  (via 16 small identity-matmuls vs local Z)
  YT_c  = matmul(lhsT=Wb,    rhs=XT_c)   -> [H, nodes_c] = next hT_c

v2 changes vs the first working version:
  * A_T (no diagonal) is loaded ONCE and stays resident in SBUF (saves
    ~12MB of DMA per core); the (1+eps)*h self-term is added with 16
    [128x128] identity matmuls per layer against the local node-major Z,
    which also gives the PE work while the AllGather is in flight.
  * Each layer's AllGather is split into two node-halves (host permutes
    A_T rows to match the gather order), so the X contraction over half A
    overlaps the gather of half B.
  * No on-device AllReduce: each core outputs per-partition partial scores
    [128, C]; the host sums them (part of the unshard step).  Removes the
    ~30-40us collective+tail at the end.
  * Readout weight streaming (wp0) is spread across all 4 layers to fill
    the AllGather bubbles; zcat copies moved to the scalar engine so they
    don't queue behind readout work on the vector engine.

All matmul operands are bf16 (fp32 PSUM accumulation).
"""

import numpy as np
import ml_dtypes

import concourse.bass as bass
import concourse.bacc as bacc
import concourse.tile as tile
import concourse.mybir as mybir
from concourse.bass_utils import run_bass_kernel_spmd

bf16 = ml_dtypes.bfloat16
dt = mybir.dt
AF = mybir.ActivationFunctionType
ALU = mybir.AluOpType

N_FULL, H_FULL, C, NL, NCORES = 4096, 512, 2, 5, 8
NLAY = NL - 1  # 4 GIN layers


def build_program(N=N_FULL, H=H_FULL, ncores=NCORES, reps=1, use_coll=True,
                  use_readout=True):
    """Emit the SPMD Bass program (same program on all cores).

    reps > 1 repeats the whole computation (for slope-based timing)."""
    NPC = N // ncores          # nodes per core
    KT0 = N // 128             # k-tiles for layer-0 MLP / adjacency contraction
    HT = H // 128              # tiles over hidden dim
    MT = NPC // 128            # tiles over this core's nodes
    NSLOT = KT0 + NLAY * HT    # readout accumulator slots per class
    KH = KT0 // 2              # k-tiles per AllGather half

    nc = bacc.Bacc("TRN2", target_bir_lowering=False, debug=False,
                   num_devices=ncores)

    featT = nc.dram_tensor("featT", [N, NPC], dt.bfloat16, kind="ExternalInput")
    w0a = nc.dram_tensor("w0a", [N, H], dt.bfloat16, kind="ExternalInput")
    wra = nc.dram_tensor("wra", [NLAY - 1, H, H], dt.bfloat16, kind="ExternalInput")
    wb = nc.dram_tensor("wb", [NLAY, H, H], dt.bfloat16, kind="ExternalInput")
    at = nc.dram_tensor("at", [N, NPC], dt.bfloat16, kind="ExternalInput")
    identl = nc.dram_tensor("identl", [NLAY * 128, 128], dt.bfloat16, kind="ExternalInput")
    wp0 = nc.dram_tensor("wp0", [C, N, NPC], dt.bfloat16, kind="ExternalInput")
    wpr = nc.dram_tensor("wpr", [NLAY, C, H, NPC], dt.bfloat16, kind="ExternalInput")
    aff = nc.dram_tensor("aff", [128, NLAY * 6 * HT], dt.float32, kind="ExternalInput")
    zdummy = (None if use_coll else
              nc.dram_tensor("zdummy", [N // 2, H], dt.bfloat16, kind="ExternalInput"))
    score_part = nc.dram_tensor("score_part", [128, C], dt.float32, kind="ExternalOutput")

    rg = [list(range(ncores))]

    def aff_col(lay, stage, m):
        return lay * 6 * HT + stage * HT + m

    with tile.TileContext(nc) as tc:
        with (
            tc.tile_pool(name="dram", bufs=2, space="DRAM") as dram,
            tc.tile_pool(name="big", bufs=1) as big,
            tc.tile_pool(name="sb", bufs=2) as sb,
            tc.tile_pool(name="stream", bufs=4) as stream,
            tc.tile_pool(name="acc", bufs=8, space="PSUM") as psum,
        ):
            # -- constants / small resident tensors
            aff_sb = big.tile([128, NLAY * 6 * HT], dt.float32, tag="aff")
            nc.sync.dma_start(aff_sb[:], aff[:])
            identl_sb = big.tile([128, NLAY, 128], dt.bfloat16, tag="identl")
            nc.sync.dma_start(
                identl_sb[:], identl[:].rearrange("(l p) c -> p l c", p=128))
            racc = big.tile([128, C * NSLOT], dt.float32, tag="racc")
            if not use_readout:
                nc.vector.memset(racc[:], 0.0)

            # -- resident tensors streamed in chunk-wise on first use
            CH = 8  # k-tiles per DMA chunk
            featT_sb = big.tile([128, KT0, NPC], dt.bfloat16, tag="featT")
            at_sb = big.tile([128, KT0, NPC], dt.bfloat16, tag="at_sb")

            def load_featT_chunk(k0):
                cc = min(CH, KT0 - k0)
                nc.sync.dma_start(
                    featT_sb[:, k0:k0 + cc, :],
                    featT[k0 * 128:(k0 + cc) * 128, :].rearrange("(t p) h -> p t h", p=128))

            def load_at_chunk(k0):
                cc = min(CH, KT0 - k0)
                nc.sync.dma_start(
                    at_sb[:, k0:k0 + cc, :],
                    at[k0 * 128:(k0 + cc) * 128, :].rearrange("(t p) h -> p t h", p=128))

            for rep in range(reps):
              # feat readout quarter q: k-tiles [q*KH/2, ...) for both classes
              def emit_feat_readout(q):
                  for c in range(C if use_readout else 0):
                      lo = q * (KT0 // NLAY)
                      hi = lo + KT0 // NLAY
                      for k0 in range(lo, hi, CH):
                          cc = min(CH, hi - k0)
                          wt = stream.tile([128, CH, NPC], dt.bfloat16, tag="wro",
                                           bufs=2, name="wt")
                          nc.sync.dma_start(
                              wt[:, :cc, :],
                              wp0[c, k0 * 128:(k0 + cc) * 128, :].rearrange("(t p) h -> p t h", p=128))
                          for kk in range(cc):
                              k = k0 + kk
                              scr = stream.tile([128, NPC], dt.float32, tag="scr",
                                                name="scr")
                              nc.vector.scalar_tensor_tensor(
                                  out=scr[:], in0=featT_sb[:, k, :], scalar=1.0,
                                  in1=wt[:, kk, :], op0=ALU.mult, op1=ALU.mult,
                                  accum_out=racc[:, c * NSLOT + k: c * NSLOT + k + 1])

              hT_sb = None
              for lay in range(NLAY):
                  K = N if lay == 0 else H
                  KT = K // 128

                  # ---- prefetch Wb for this layer (lands during AG wait) ----
                  wb_sb = sb.tile([128, HT, H], dt.bfloat16, tag="wb")
                  nc.sync.dma_start(
                      wb_sb[:], wb[lay].rearrange("(t p) h -> p t h", p=128))

                  # ---- Z_c = h_c @ Wa  (node-major out [NPC, H]) ----
                  psZ = [psum.tile([128, H], dt.float32, tag="acc", name=f"psZ{m}") for m in range(MT)]
                  wsrc = w0a if lay == 0 else wra[lay - 1]
                  for k0 in range(0, KT, CH):
                      cc = min(CH, KT - k0)
                      if lay == 0 and rep == 0:
                          load_featT_chunk(k0)
                      wa_t = stream.tile([128, CH, H], dt.bfloat16, tag="wa", bufs=2)
                      nc.sync.dma_start(
                          wa_t[:, :cc, :],
                          wsrc[k0 * 128:(k0 + cc) * 128, :].rearrange("(t p) h -> p t h", p=128))
                      for kk in range(cc):
                          k = k0 + kk
                          lhs_tile = featT_sb[:, k, :] if lay == 0 else hT_sb[:, k, :]
                          for m in range(MT):
                              nc.tensor.matmul(
                                  psZ[m][:], lhsT=lhs_tile[:, m * 128:(m + 1) * 128],
                                  rhs=wa_t[:, kk, :], start=(k == 0), stop=(k == KT - 1))

                  # ---- node-major Z copy (scalar engine; bf16 cast) ----
                  zcat = sb.tile([128, MT, H], dt.bfloat16, tag="zcat")
                  for m in range(MT):
                      nc.scalar.activation(zcat[:, m, :], psZ[m][:], AF.Copy)

                  # ---- AllGather Z in two node-halves ----
                  zsrcs = []
                  for half in range(2):
                      zin_h = dram.tile([NPC // 2, H], dt.bfloat16, tag=f"zin{half}")
                      nc.sync.dma_start(
                          zin_h.rearrange("(m p) h -> p m h", p=128),
                          zcat[:, 2 * half:2 * half + 2, :])
                      if use_coll:
                          zfull_h = dram.tile([N // 2, H], dt.bfloat16,
                                              tag=f"zfull{half}", addr_space="Shared")
                          nc.gpsimd.collective_compute(
                              "AllGather", ALU.bypass, replica_groups=rg,
                              ins=[zin_h.opt()], outs=[zfull_h.opt()])
                          zsrcs.append(zfull_h)
                      else:
                          zsrcs.append(zdummy)

                  # ---- XT_c = Zfull.T @ A_T_c + (1+eps) Z_c^T -> [H, NPC] ----
                  psX = [psum.tile([128, NPC], dt.float32, tag="acc", name=f"psX{m}") for m in range(HT)]
                  # (1+eps) Z_c^T from local node-major Z; runs during the AG.
                  # mn=0's start=True marks the whole PSUM bank pending-zero;
                  # mn=1..3 overwrite their (still-pending) column blocks.
                  for mn in range(MT):
                      for m in range(HT):
                          nc.tensor.matmul(
                              psX[m][:, mn * 128:(mn + 1) * 128],
                              lhsT=zcat[:, mn, m * 128:(m + 1) * 128],
                              rhs=identl_sb[:, lay, :],
                              start=(mn == 0), stop=False)

                  zf_sb = sb.tile([128, KT0, H], dt.bfloat16, tag="zf", bufs=1)
                  for half in range(2):
                      if lay == 0 and rep == 0:
                          for k0 in range(half * KH, (half + 1) * KH, CH):
                              load_at_chunk(k0)
                      for t0 in range(0, KH, CH):
                          cc = min(CH, KH - t0)
                          nc.sync.dma_start(
                              zf_sb[:, half * KH + t0:half * KH + t0 + cc, :],
                              zsrcs[half][t0 * 128:(t0 + cc) * 128, :].rearrange("(t p) h -> p t h", p=128))
                      for k in range(half * KH, (half + 1) * KH):
                          for m in range(HT):
                              nc.tensor.matmul(
                                  psX[m][:], lhsT=zf_sb[:, k, m * 128:(m + 1) * 128],
                                  rhs=at_sb[:, k, :], start=False, stop=(k == KT0 - 1))

                  # ---- stage 1: relu(X * s1 + t1), cast bf16 ----
                  xt_sb = sb.tile([128, HT, NPC], dt.bfloat16, tag="xt")
                  for m in range(HT):
                      nc.scalar.activation(
                          xt_sb[:, m, :], psX[m][:], AF.Relu,
                          bias=aff_sb[:, aff_col(lay, 1, m):aff_col(lay, 1, m) + 1],
                          scale=aff_sb[:, aff_col(lay, 0, m):aff_col(lay, 0, m) + 1])

                  # ---- YT_c = Wb.T @ XT_c -> [H, NPC] ----
                  psY = [psum.tile([128, NPC], dt.float32, tag="acc", name=f"psY{m}") for m in range(HT)]
                  for k in range(HT):
                      for m in range(HT):
                          nc.tensor.matmul(
                              psY[m][:], lhsT=wb_sb[:, k, m * 128:(m + 1) * 128],
                              rhs=xt_sb[:, k, :], start=(k == 0), stop=(k == HT - 1))

                  # ---- stages 2+3: two fused affine+relu, cast bf16 ----
                  hT_sb = sb.tile([128, HT, NPC], dt.bfloat16, tag="hT")
                  for m in range(HT):
                      tmp = sb.tile([128, NPC], dt.float32, tag="tmp")
                      nc.scalar.activation(
                          tmp[:], psY[m][:], AF.Relu,
                          bias=aff_sb[:, aff_col(lay, 3, m):aff_col(lay, 3, m) + 1],
                          scale=aff_sb[:, aff_col(lay, 2, m):aff_col(lay, 2, m) + 1])
                      nc.scalar.activation(
                          hT_sb[:, m, :], tmp[:], AF.Relu,
                          bias=aff_sb[:, aff_col(lay, 5, m):aff_col(lay, 5, m) + 1],
                          scale=aff_sb[:, aff_col(lay, 4, m):aff_col(lay, 4, m) + 1])

                  # ---- readout for this hidden rep ----
                  for c in range(C if use_readout else 0):
                      wt = stream.tile([128, HT, NPC], dt.bfloat16, tag="wrr", bufs=2)
                      nc.sync.dma_start(
                          wt[:], wpr[lay, c].rearrange("(t p) h -> p t h", p=128))
                      for m in range(HT):
                          scr = stream.tile([128, NPC], dt.float32, tag="scr")
                          slot = c * NSLOT + KT0 + lay * HT + m
                          nc.vector.scalar_tensor_tensor(
                              out=scr[:], in0=hT_sb[:, m, :], scalar=1.0,
                              in1=wt[:, m, :], op0=ALU.mult, op1=ALU.mult,
                              accum_out=racc[:, slot:slot + 1])

                  # feat readout quarter for this layer (fills AG bubbles)
                  emit_feat_readout(lay)

              # ---- finish readout: free-dim reduce; partition sum on host ----
              r2 = sb.tile([128, C], dt.float32, tag="r2")
              for c in range(C):
                  nc.vector.tensor_reduce(
                      r2[:, c:c + 1], racc[:, c * NSLOT:(c + 1) * NSLOT],
                      axis=mybir.AxisListType.X, op=ALU.add)
              nc.sync.dma_start(score_part[:], r2[:])

    nc.compile()
    return nc


def prep_inputs(inputs, N=N_FULL, H=H_FULL, ncores=NCORES, nlay=NLAY):
    """Host-side re-layout of the full inputs into per-core input maps.

    Returns (in_maps, bias_tot)."""
    inp = {k: np.asarray(v) for k, v in inputs.items()}
    NPC = N // ncores
    HT = H // 128
    f32 = np.float32

    feat = inp["feat"].astype(f32)
    src = inp["edge_src"].astype(np.int64)
    dst = inp["edge_dst"].astype(np.int64)

    A_T = np.zeros((N, N), f32)
    np.add.at(A_T, (src, dst), 1.0)
    eps_list = [float(inp["eps0"])] + [float(x) for x in inp["epsR"]]

    # Row order matching the two-half AllGather: half g of every core's
    # node block, cores in rank order.
    half = NPC // 2
    r = np.arange(N // 2)
    perm = np.concatenate([(r // half) * NPC + (r % half),
                           (r // half) * NPC + half + (r % half)])
    at_perm = np.ascontiguousarray(A_T[perm]).astype(bf16)

    identl = np.zeros((nlay * 128, 128), f32)
    for i in range(nlay):
        identl[i * 128:(i + 1) * 128] = (1.0 + eps_list[i]) * np.eye(128, dtype=f32)
    identl = identl.astype(bf16)

    featT = np.ascontiguousarray(feat.T).astype(bf16)
    w0a = inp["W0a"].astype(f32).astype(bf16)
    wra = inp["WRa"].astype(f32).astype(bf16)
    wb = np.concatenate([inp["W0b"][None], inp["WRb"]], axis=0).astype(f32).astype(bf16)

    ba = [inp["b0a"]] + [inp["bRa"][i] for i in range(nlay - 1)]
    bb = [inp["b0b"]] + [inp["bRb"][i] for i in range(nlay - 1)]

    def fold(nm, i):
        idx = (lambda x: x) if i == 0 else (lambda x: x[i - 1])
        g, b_, m, v = [idx(inp[nm + s]) for s in ("_g", "_b", "_m", "_v")]
        s = (g / np.sqrt(v + 1e-5)).astype(f32)
        return s, b_, m

    # aff[p, lay*6*HT + stage*HT + m] with stages (s1,t1,s2,t2,s3,t3)
    aff = np.zeros((128, nlay * 6 * HT), f32)
    for i in range(nlay):
        nms = ("bn0a", "bnA0", "bnO0") if i == 0 else ("bnRa", "bnAR", "bnOR")
        s, b_, m = fold(nms[0], i)
        p1s, p1t = s, ((ba[i] - m) * s + b_).astype(f32)
        s, b_, m = fold(nms[1], i)
        p2s, p2t = s, ((bb[i] - m) * s + b_).astype(f32)
        s, b_, m = fold(nms[2], i)
        p3s, p3t = s, (b_ - m * s).astype(f32)
        for mi in range(HT):
            sl = slice(mi * 128, (mi + 1) * 128)
            for j, vec in enumerate((p1s, p1t, p2s, p2t, p3s, p3t)):
                aff[:, i * 6 * HT + j * HT + mi] = vec[sl]

    wp0r = np.ascontiguousarray(
        inp["Wp0"].astype(f32).reshape(N, N, C).transpose(2, 1, 0)).astype(bf16)
    wprr = np.ascontiguousarray(
        inp["WpR"].astype(f32).reshape(nlay, N, H, C).transpose(0, 3, 2, 1)).astype(bf16)
    bias_tot = (inp["bp0"] + inp["bpR"].sum(axis=0)).astype(f32).reshape(1, 2)

    in_maps = []
    for cix in range(ncores):
        sl = slice(cix * NPC, (cix + 1) * NPC)
        in_maps.append({
            "featT": np.ascontiguousarray(featT[:, sl]),
            "w0a": w0a,
            "wra": wra,
            "wb": wb,
            "at": np.ascontiguousarray(at_perm[:, sl]),
            "identl": identl,
            "wp0": np.ascontiguousarray(wp0r[:, :, sl]),
            "wpr": np.ascontiguousarray(wprr[:, :, :, sl]),
            "aff": aff,
        })
    return in_maps, bias_tot


_CACHE = {}


def _get_program():
    if "nc" not in _CACHE:
        _CACHE["nc"] = build_program()
    return _CACHE["nc"]


def kernel(**inputs):
    nc = _get_program()
    in_maps, bias_tot = prep_inputs(inputs)
    res = run_bass_kernel_spmd(nc, in_maps, list(range(NCORES)))
    acc = np.zeros((1, C), np.float64)
    for cix in range(NCORES):
        acc += res.results[cix]["score_part"].astype(np.float64).sum(axis=0)
    return (acc + bias_tot).astype(np.float32)


# revision 4
# speedup vs baseline: 8.2028x; 8.2028x over previous
"""GIN message-passing network on 8 Trainium2 NeuronCores.

Strategy
--------
The edge list is converted (on host, O(E)) to a dense transposed adjacency
matrix A_T[src, dst].  segment_sum(h[src], dst) == A @ h, and since the GIN
update is ((1+eps)h + A@h) @ W = (1+eps)(h@W) + A@(h@W), the N x N adjacency
contraction runs at hidden width H=512 instead of 4096.  BatchNorm (eval
mode) + bias are folded on host into per-feature affine scale/shift applied
with fused scalar-engine relu-activations.

Sharding: nodes are split 8 ways (512 nodes per core).  Activations are kept
feature-major ("transposed", [H, nodes]) on each core so that per-feature
affines land on the partition axis and no on-device transposes are needed:

  Z_c   = matmul(lhsT=hT_c,  rhs=Wa)     -> [nodes_c, H] (node-major)
  AllGather(Z_c)                         -> Zfull [N, H]
  XT_c  = matmul(lhsT=Zfull, rhs=AT_c)   -> [H, nodes_c]
        + (1+eps) * Z_c^T  (via 16 small identity-matmuls vs local Z)
  YT_c  = matmul(lhsT=Wb,    rhs=XT_c)   -> [H, nodes_c] = next hT_c

v2 changes vs the first working version:
  * A_T (no diagonal) is loaded ONCE and stays resident in SBUF (saves
    ~12MB of DMA per core); the (1+eps)*h self-term is added with 16
    [128x128] identity matmuls per layer against the local node-major Z,
    which also gives the PE work while the AllGather is in flight.
  * Each layer's AllGather is split into two node-halves (host permutes
    A_T rows to match the gather order), so the X contraction over half A
    overlaps the gather of half B.
  * No on-device AllReduce: each core outputs per-partition partial scores
    [128, C]; the host sums them (part of the unshard step).  Removes the
    ~30-40us collective+tail at the end.
  * Readout weight streaming (wp0) is spread across all 4 layers to fill
    the AllGather bubbles; zcat copies moved to the scalar engine so they
    don't queue behind readout work on the vector engine.

All matmul operands are bf16 (fp32 PSUM accumulation).
"""

import numpy as np
import ml_dtypes

import concourse.bass as bass
import concourse.bacc as bacc
import concourse.tile as tile
import concourse.mybir as mybir
from concourse.bass_utils import run_bass_kernel_spmd

bf16 = ml_dtypes.bfloat16
dt = mybir.dt
AF = mybir.ActivationFunctionType
ALU = mybir.AluOpType

N_FULL, H_FULL, C, NL, NCORES = 4096, 512, 2, 5, 8
NLAY = NL - 1  # 4 GIN layers


def build_program(N=N_FULL, H=H_FULL, ncores=NCORES, reps=1, use_coll=True,
                  use_readout=True):
    """Emit the SPMD Bass program (same program on all cores).

    reps > 1 repeats the whole computation (for slope-based timing)."""
    NPC = N // ncores          # nodes per core
    KT0 = N // 128             # k-tiles for layer-0 MLP / adjacency contraction
    HT = H // 128              # tiles over hidden dim
    MT = NPC // 128            # tiles over this core's nodes
    NSLOT = KT0 + NLAY * HT    # readout accumulator slots per class
    KH = KT0 // 2              # k-tiles per AllGather half

    nc = bacc.Bacc("TRN2", target_bir_lowering=False, debug=False,
                   num_devices=ncores)

    featT = nc.dram_tensor("featT", [N, NPC], dt.bfloat16, kind="ExternalInput")
    w0a = nc.dram_tensor("w0a", [N, H], dt.bfloat16, kind="ExternalInput")
    wra = nc.dram_tensor("wra", [NLAY - 1, H, H], dt.bfloat16, kind="ExternalInput")
    wb = nc.dram_tensor("wb", [NLAY, H, H], dt.bfloat16, kind="ExternalInput")
    at = nc.dram_tensor("at", [N, NPC], dt.bfloat16, kind="ExternalInput")
    identl = nc.dram_tensor("identl", [NLAY * 128, 128], dt.bfloat16, kind="ExternalInput")
    wp0 = nc.dram_tensor("wp0", [C, N, NPC], dt.bfloat16, kind="ExternalInput")
    wpr = nc.dram_tensor("wpr", [NLAY, C, H, NPC], dt.bfloat16, kind="ExternalInput")
    aff = nc.dram_tensor("aff", [128, NLAY * 6 * HT], dt.float32, kind="ExternalInput")
    zdummy = (None if use_coll else
              nc.dram_tensor("zdummy", [N // 2, H], dt.bfloat16, kind="ExternalInput"))
    score_part = nc.dram_tensor("score_part", [128, C], dt.float32, kind="ExternalOutput")

    rg = [list(range(ncores))]

    def aff_col(lay, stage, m):
        return lay * 6 * HT + stage * HT + m

    with tile.TileContext(nc) as tc:
        with (
            tc.tile_pool(name="dram", bufs=2, space="DRAM") as dram,
            tc.tile_pool(name="big", bufs=1) as big,
            tc.tile_pool(name="sb", bufs=2) as sb,
            tc.tile_pool(name="stream", bufs=4) as stream,
            tc.tile_pool(name="acc", bufs=8, space="PSUM") as psum,
        ):
            # -- constants / small resident tensors
            aff_sb = big.tile([128, NLAY * 6 * HT], dt.float32, tag="aff")
            nc.sync.dma_start(aff_sb[:], aff[:])
            identl_sb = big.tile([128, NLAY, 128], dt.bfloat16, tag="identl")
            nc.sync.dma_start(
                identl_sb[:], identl[:].rearrange("(l p) c -> p l c", p=128))
            racc = big.tile([128, C * NSLOT], dt.float32, tag="racc")
            if not use_readout:
                nc.vector.memset(racc[:], 0.0)

            # -- resident tensors streamed in chunk-wise on first use
            CH = 8  # k-tiles per DMA chunk
            featT_sb = big.tile([128, KT0, NPC], dt.bfloat16, tag="featT")
            at_sb = big.tile([128, KT0, NPC], dt.bfloat16, tag="at_sb")

            def load_featT_chunk(k0):
                cc = min(CH, KT0 - k0)
                nc.sync.dma_start(
                    featT_sb[:, k0:k0 + cc, :],
                    featT[k0 * 128:(k0 + cc) * 128, :].rearrange("(t p) h -> p t h", p=128))

            def load_at_chunk(k0):
                cc = min(CH, KT0 - k0)
                nc.sync.dma_start(
                    at_sb[:, k0:k0 + cc, :],
                    at[k0 * 128:(k0 + cc) * 128, :].rearrange("(t p) h -> p t h", p=128))

            for rep in range(reps):
              # feat readout quarter q: k-tiles [q*KH/2, ...) for both classes
              def emit_feat_readout(q):
                  for c in range(C if use_readout else 0):
                      lo = q * (KT0 // NLAY)
                      hi = lo + KT0 // NLAY
                      for k0 in range(lo, hi, CH):
                          cc = min(CH, hi - k0)
                          wt = stream.tile([128, CH, NPC], dt.bfloat16, tag="wro",
                                           bufs=2, name="wt")
                          nc.sync.dma_start(
                              wt[:, :cc, :],
                              wp0[c, k0 * 128:(k0 + cc) * 128, :].rearrange("(t p) h -> p t h", p=128))
                          for kk in range(cc):
                              k = k0 + kk
                              scr = stream.tile([128, NPC], dt.float32, tag="scr",
                                                name="scr")
                              nc.vector.scalar_tensor_tensor(
                                  out=scr[:], in0=featT_sb[:, k, :], scalar=1.0,
                                  in1=wt[:, kk, :], op0=ALU.mult, op1=ALU.mult,
                                  accum_out=racc[:, c * NSLOT + k: c * NSLOT + k + 1])

              hT_sb = None
              for lay in range(NLAY):
                  K = N if lay == 0 else H
                  KT = K // 128

                  # ---- prefetch Wb for this layer (lands during AG wait) ----
                  wb_sb = sb.tile([128, HT, H], dt.bfloat16, tag="wb")
                  nc.sync.dma_start(
                      wb_sb[:], wb[lay].rearrange("(t p) h -> p t h", p=128))

                  # ---- Z_c = h_c @ Wa  (node-major out [NPC, H]) ----
                  psZ = [psum.tile([128, H], dt.float32, tag="acc", name=f"psZ{m}") for m in range(MT)]
                  wsrc = w0a if lay == 0 else wra[lay - 1]
                  for k0 in range(0, KT, CH):
                      cc = min(CH, KT - k0)
                      if lay == 0 and rep == 0:
                          load_featT_chunk(k0)
                      wa_t = stream.tile([128, CH, H], dt.bfloat16, tag="wa", bufs=2)
                      nc.sync.dma_start(
                          wa_t[:, :cc, :],
                          wsrc[k0 * 128:(k0 + cc) * 128, :].rearrange("(t p) h -> p t h", p=128))
                      for kk in range(cc):
                          k = k0 + kk
                          lhs_tile = featT_sb[:, k, :] if lay == 0 else hT_sb[:, k, :]
                          for m in range(MT):
                              nc.tensor.matmul(
                                  psZ[m][:], lhsT=lhs_tile[:, m * 128:(m + 1) * 128],
                                  rhs=wa_t[:, kk, :], start=(k == 0), stop=(k == KT - 1))

                  # ---- node-major Z copy (scalar engine; bf16 cast) ----
                  zcat = sb.tile([128, MT, H], dt.bfloat16, tag="zcat")
                  for m in range(MT):
                      nc.scalar.activation(zcat[:, m, :], psZ[m][:], AF.Copy)

                  # ---- AllGather Z in two node-halves ----
                  zsrcs = []
                  for half in range(2):
                      zin_h = dram.tile([NPC // 2, H], dt.bfloat16, tag=f"zin{half}")
                      nc.sync.dma_start(
                          zin_h.rearrange("(m p) h -> p m h", p=128),
                          zcat[:, 2 * half:2 * half + 2, :])
                      if use_coll:
                          zfull_h = dram.tile([N // 2, H], dt.bfloat16,
                                              tag=f"zfull{half}", addr_space="Shared")
                          nc.gpsimd.collective_compute(
                              "AllGather", ALU.bypass, replica_groups=rg,
                              ins=[zin_h.opt()], outs=[zfull_h.opt()])
                          zsrcs.append(zfull_h)
                      else:
                          zsrcs.append(zdummy)

                  # ---- XT_c = Zfull.T @ A_T_c + (1+eps) Z_c^T -> [H, NPC] ----
                  psX = [psum.tile([128, NPC], dt.float32, tag="acc", name=f"psX{m}") for m in range(HT)]
                  # (1+eps) Z_c^T from local node-major Z; runs during the AG.
                  # mn=0's start=True marks the whole PSUM bank pending-zero;
                  # mn=1..3 overwrite their (still-pending) column blocks.
                  for mn in range(MT):
                      for m in range(HT):
                          nc.tensor.matmul(
                              psX[m][:, mn * 128:(mn + 1) * 128],
                              lhsT=zcat[:, mn, m * 128:(m + 1) * 128],
                              rhs=identl_sb[:, lay, :],
                              start=(mn == 0), stop=False)

                  # prefetch this layer's readout weights during the AG wait
                  wrr_tiles = []
                  for c in range(C if use_readout else 0):
                      wt = stream.tile([128, HT, NPC], dt.bfloat16, tag="wrr", bufs=2)
                      nc.sync.dma_start(
                          wt[:], wpr[lay, c].rearrange("(t p) h -> p t h", p=128))
                      wrr_tiles.append(wt)

                  zf_sb = sb.tile([128, KT0, H], dt.bfloat16, tag="zf", bufs=1)
                  for half in range(2):
                      if lay == 0 and rep == 0:
                          for k0 in range(half * KH, (half + 1) * KH, CH):
                              load_at_chunk(k0)
                      # small first chunk so the X matmuls start sooner
                      for t0, cc in ((0, 4), (4, 8), (12, 4)):
                          nc.sync.dma_start(
                              zf_sb[:, half * KH + t0:half * KH + t0 + cc, :],
                              zsrcs[half][t0 * 128:(t0 + cc) * 128, :].rearrange("(t p) h -> p t h", p=128))
                      for k in range(half * KH, (half + 1) * KH):
                          for m in range(HT):
                              nc.tensor.matmul(
                                  psX[m][:], lhsT=zf_sb[:, k, m * 128:(m + 1) * 128],
                                  rhs=at_sb[:, k, :], start=False, stop=(k == KT0 - 1))

                  # ---- stage 1: relu(X * s1 + t1), cast bf16 ----
                  xt_sb = sb.tile([128, HT, NPC], dt.bfloat16, tag="xt")
                  for m in range(HT):
                      nc.scalar.activation(
                          xt_sb[:, m, :], psX[m][:], AF.Relu,
                          bias=aff_sb[:, aff_col(lay, 1, m):aff_col(lay, 1, m) + 1],
                          scale=aff_sb[:, aff_col(lay, 0, m):aff_col(lay, 0, m) + 1])

                  # ---- YT_c = Wb.T @ XT_c -> [H, NPC] ----
                  psY = [psum.tile([128, NPC], dt.float32, tag="acc", name=f"psY{m}") for m in range(HT)]
                  for k in range(HT):
                      for m in range(HT):
                          nc.tensor.matmul(
                              psY[m][:], lhsT=wb_sb[:, k, m * 128:(m + 1) * 128],
                              rhs=xt_sb[:, k, :], start=(k == 0), stop=(k == HT - 1))

                  # ---- stages 2+3: two fused affine+relu, cast bf16 ----
                  hT_sb = sb.tile([128, HT, NPC], dt.bfloat16, tag="hT")
                  for m in range(HT):
                      tmp = sb.tile([128, NPC], dt.float32, tag="tmp")
                      nc.scalar.activation(
                          tmp[:], psY[m][:], AF.Relu,
                          bias=aff_sb[:, aff_col(lay, 3, m):aff_col(lay, 3, m) + 1],
                          scale=aff_sb[:, aff_col(lay, 2, m):aff_col(lay, 2, m) + 1])
                      nc.scalar.activation(
                          hT_sb[:, m, :], tmp[:], AF.Relu,
                          bias=aff_sb[:, aff_col(lay, 5, m):aff_col(lay, 5, m) + 1],
                          scale=aff_sb[:, aff_col(lay, 4, m):aff_col(lay, 4, m) + 1])

                  # ---- readout for this hidden rep ----
                  for c in range(C if use_readout else 0):
                      wt = wrr_tiles[c]
                      for m in range(HT):
                          scr = stream.tile([128, NPC], dt.float32, tag="scr")
                          slot = c * NSLOT + KT0 + lay * HT + m
                          nc.vector.scalar_tensor_tensor(
                              out=scr[:], in0=hT_sb[:, m, :], scalar=1.0,
                              in1=wt[:, m, :], op0=ALU.mult, op1=ALU.mult,
                              accum_out=racc[:, slot:slot + 1])

                  # feat readout quarters (fill AG bubbles); quarter 3 is
                  # emitted with layer 2 so the last layer has no DVE tail
                  for q in {0: [0], 1: [1], 2: [2, 3], 3: []}[lay]:
                      emit_feat_readout(q)

              # ---- finish readout: free-dim reduce; partition sum on host ----
              r2 = sb.tile([128, C], dt.float32, tag="r2")
              for c in range(C):
                  nc.vector.tensor_reduce(
                      r2[:, c:c + 1], racc[:, c * NSLOT:(c + 1) * NSLOT],
                      axis=mybir.AxisListType.X, op=ALU.add)
              nc.sync.dma_start(score_part[:], r2[:])

    nc.compile()
    return nc


def prep_inputs(inputs, N=N_FULL, H=H_FULL, ncores=NCORES, nlay=NLAY):
    """Host-side re-layout of the full inputs into per-core input maps.

    Returns (in_maps, bias_tot)."""
    inp = {k: np.asarray(v) for k, v in inputs.items()}
    NPC = N // ncores
    HT = H // 128
    f32 = np.float32

    feat = inp["feat"].astype(f32)
    src = inp["edge_src"].astype(np.int64)
    dst = inp["edge_dst"].astype(np.int64)

    A_T = np.zeros((N, N), f32)
    np.add.at(A_T, (src, dst), 1.0)
    eps_list = [float(inp["eps0"])] + [float(x) for x in inp["epsR"]]

    # Row order matching the two-half AllGather: half g of every core's
    # node block, cores in rank order.
    half = NPC // 2
    r = np.arange(N // 2)
    perm = np.concatenate([(r // half) * NPC + (r % half),
                           (r // half) * NPC + half + (r % half)])
    at_perm = np.ascontiguousarray(A_T[perm]).astype(bf16)

    identl = np.zeros((nlay * 128, 128), f32)
    for i in range(nlay):
        identl[i * 128:(i + 1) * 128] = (1.0 + eps_list[i]) * np.eye(128, dtype=f32)
    identl = identl.astype(bf16)

    featT = np.ascontiguousarray(feat.T).astype(bf16)
    w0a = inp["W0a"].astype(f32).astype(bf16)
    wra = inp["WRa"].astype(f32).astype(bf16)
    wb = np.concatenate([inp["W0b"][None], inp["WRb"]], axis=0).astype(f32).astype(bf16)

    ba = [inp["b0a"]] + [inp["bRa"][i] for i in range(nlay - 1)]
    bb = [inp["b0b"]] + [inp["bRb"][i] for i in range(nlay - 1)]

    def fold(nm, i):
        idx = (lambda x: x) if i == 0 else (lambda x: x[i - 1])
        g, b_, m, v = [idx(inp[nm + s]) for s in ("_g", "_b", "_m", "_v")]
        s = (g / np.sqrt(v + 1e-5)).astype(f32)
        return s, b_, m

    # aff[p, lay*6*HT + stage*HT + m] with stages (s1,t1,s2,t2,s3,t3)
    aff = np.zeros((128, nlay * 6 * HT), f32)
    for i in range(nlay):
        nms = ("bn0a", "bnA0", "bnO0") if i == 0 else ("bnRa", "bnAR", "bnOR")
        s, b_, m = fold(nms[0], i)
        p1s, p1t = s, ((ba[i] - m) * s + b_).astype(f32)
        s, b_, m = fold(nms[1], i)
        p2s, p2t = s, ((bb[i] - m) * s + b_).astype(f32)
        s, b_, m = fold(nms[2], i)
        p3s, p3t = s, (b_ - m * s).astype(f32)
        for mi in range(HT):
            sl = slice(mi * 128, (mi + 1) * 128)
            for j, vec in enumerate((p1s, p1t, p2s, p2t, p3s, p3t)):
                aff[:, i * 6 * HT + j * HT + mi] = vec[sl]

    wp0r = np.ascontiguousarray(
        inp["Wp0"].astype(f32).reshape(N, N, C).transpose(2, 1, 0)).astype(bf16)
    wprr = np.ascontiguousarray(
        inp["WpR"].astype(f32).reshape(nlay, N, H, C).transpose(0, 3, 2, 1)).astype(bf16)
    bias_tot = (inp["bp0"] + inp["bpR"].sum(axis=0)).astype(f32).reshape(1, 2)

    in_maps = []
    for cix in range(ncores):
        sl = slice(cix * NPC, (cix + 1) * NPC)
        in_maps.append({
            "featT": np.ascontiguousarray(featT[:, sl]),
            "w0a": w0a,
            "wra": wra,
            "wb": wb,
            "at": np.ascontiguousarray(at_perm[:, sl]),
            "identl": identl,
            "wp0": np.ascontiguousarray(wp0r[:, :, sl]),
            "wpr": np.ascontiguousarray(wprr[:, :, :, sl]),
            "aff": aff,
        })
    return in_maps, bias_tot


_CACHE = {}


def _get_program():
    if "nc" not in _CACHE:
        _CACHE["nc"] = build_program()
    return _CACHE["nc"]


def kernel(**inputs):
    nc = _get_program()
    in_maps, bias_tot = prep_inputs(inputs)
    res = run_bass_kernel_spmd(nc, in_maps, list(range(NCORES)))
    acc = np.zeros((1, C), np.float64)
    for cix in range(NCORES):
        acc += res.results[cix]["score_part"].astype(np.float64).sum(axis=0)
    return (acc + bias_tot).astype(np.float32)


# revision 18
# speedup vs baseline: 9.9968x; 1.2187x over previous
"""GIN message-passing network on 8 Trainium2 NeuronCores.

Strategy
--------
The edge list is converted (on host, O(E)) to a dense transposed adjacency
matrix A_T[src, dst].  segment_sum(h[src], dst) == A @ h, and since the GIN
update is ((1+eps)h + A@h) @ W = (1+eps)(h@W) + A@(h@W), the N x N adjacency
contraction runs at hidden width H=512 instead of 4096.  BatchNorm (eval
mode) + bias are folded on host into per-feature affine scale/shift applied
with fused scalar-engine relu-activations.

Sharding: nodes are split 8 ways (512 nodes per core).  Activations are kept
feature-major ("transposed", [H, nodes]) on each core so that per-feature
affines land on the partition axis and no on-device transposes are needed:

  Z_c   = matmul(lhsT=hT_c,  rhs=Wa)     -> [nodes_c, H] (node-major)
  AllGather(Z_c)                         -> Zfull [N, H]
  XT_c  = matmul(lhsT=Zfull, rhs=AT_c)   -> [H, nodes_c]
        + (1+eps) * Z_c^T  (via 16 small identity-matmuls vs local Z)
  YT_c  = matmul(lhsT=Wb,    rhs=XT_c)   -> [H, nodes_c] = next hT_c

v2 changes vs the first working version:
  * A_T (no diagonal) is loaded ONCE and stays resident in SBUF (saves
    ~12MB of DMA per core); the (1+eps)*h self-term is added with 16
    [128x128] identity matmuls per layer against the local node-major Z,
    which also gives the PE work while the AllGather is in flight.
  * Each layer's AllGather is split into two node-halves (host permutes
    A_T rows to match the gather order), so the X contraction over half A
    overlaps the gather of half B.
  * No on-device AllReduce: each core outputs per-partition partial scores
    [128, C]; the host sums them (part of the unshard step).  Removes the
    ~30-40us collective+tail at the end.
  * Readout weight streaming (wp0) is spread across all 4 layers to fill
    the AllGather bubbles; zcat copies moved to the scalar engine so they
    don't queue behind readout work on the vector engine.

All matmul operands are bf16 (fp32 PSUM accumulation).
"""

import numpy as np
import ml_dtypes

import concourse.bass as bass
import concourse.bacc as bacc
import concourse.tile as tile
import concourse.mybir as mybir
from concourse.bass_utils import run_bass_kernel_spmd

bf16 = ml_dtypes.bfloat16
dt = mybir.dt
AF = mybir.ActivationFunctionType
ALU = mybir.AluOpType

N_FULL, H_FULL, C, NL, NCORES = 4096, 512, 2, 5, 8
NLAY = NL - 1  # 4 GIN layers


def build_program(N=N_FULL, H=H_FULL, ncores=NCORES, reps=1, use_coll=True,
                  use_readout=True):
    """Emit the SPMD Bass program (same program on all cores).

    reps > 1 repeats the whole computation (for slope-based timing)."""
    NPC = N // ncores          # nodes per core
    KT0 = N // 128             # k-tiles for layer-0 MLP / adjacency contraction
    HT = H // 128              # tiles over hidden dim
    MT = NPC // 128            # tiles over this core's nodes
    NSLOT = KT0 + NLAY * HT    # readout accumulator slots per class
    KH = KT0 // 2              # k-tiles per AllGather half

    nc = bacc.Bacc("TRN2", target_bir_lowering=False, debug=False,
                   num_devices=ncores)

    # All host-prepared tensors are pre-swizzled to partition-major
    # [128, tiles, free] layout so every DMA load is contiguous per
    # partition (no strided descriptors).
    featT = nc.dram_tensor("featT", [128, KT0, NPC], dt.bfloat16, kind="ExternalInput")
    w0a = nc.dram_tensor("w0a", [128, KT0, H], dt.bfloat16, kind="ExternalInput")
    wra = nc.dram_tensor("wra", [NLAY - 1, 128, HT, H], dt.bfloat16, kind="ExternalInput")
    wb = nc.dram_tensor("wb", [NLAY, 128, HT, H], dt.bfloat16, kind="ExternalInput")
    at = nc.dram_tensor("at", [128, KT0, NPC], dt.bfloat16, kind="ExternalInput")
    identl = nc.dram_tensor("identl", [128, NLAY, 128], dt.bfloat16, kind="ExternalInput")
    wp0 = nc.dram_tensor("wp0", [C, 128, KT0, NPC], dt.bfloat16, kind="ExternalInput")
    wpr = nc.dram_tensor("wpr", [NLAY, C, 128, HT, NPC], dt.bfloat16, kind="ExternalInput")
    aff = nc.dram_tensor("aff", [128, NLAY * 6 * HT], dt.float32, kind="ExternalInput")
    zdummy = (None if use_coll else
              nc.dram_tensor("zdummy", [N // 2, H], dt.bfloat16, kind="ExternalInput"))
    score_part = nc.dram_tensor("score_part", [128, C], dt.float32, kind="ExternalOutput")

    rg = [list(range(ncores))]

    def aff_col(lay, stage, m):
        return lay * 6 * HT + stage * HT + m

    with tile.TileContext(nc) as tc:
        with (
            tc.tile_pool(name="dram", bufs=2, space="DRAM") as dram,
            tc.tile_pool(name="big", bufs=1) as big,
            tc.tile_pool(name="sb", bufs=2) as sb,
            tc.tile_pool(name="stream", bufs=4) as stream,
            tc.tile_pool(name="acc", bufs=7, space="PSUM") as psum,
            tc.tile_pool(name="warm", bufs=1, space="PSUM") as warm,
        ):
            # -- constants / small resident tensors (loads emitted inside
            # layer 0 so they don't delay the first featT/w0a chunks)
            aff_sb = big.tile([128, NLAY * 6 * HT], dt.float32, tag="aff")
            identl_sb = big.tile([128, NLAY, 128], dt.bfloat16, tag="identl")
            racc = big.tile([128, C * NSLOT], dt.float32, tag="racc")
            if not use_readout:
                nc.vector.memset(racc[:], 0.0)

            # -- resident tensors streamed in chunk-wise on first use
            CH = 8  # k-tiles per DMA chunk
            featT_sb = big.tile([128, KT0, NPC], dt.bfloat16, tag="featT")
            at_sb = big.tile([128, KT0, NPC], dt.bfloat16, tag="at_sb")

            def load_featT_chunk(k0):
                cc = min(CH, KT0 - k0)
                nc.sync.dma_start(featT_sb[:, k0:k0 + cc, :], featT[:, k0:k0 + cc, :])

            def load_at_chunk(k0):
                cc = min(CH, KT0 - k0)
                nc.sync.dma_start(at_sb[:, k0:k0 + cc, :], at[:, k0:k0 + cc, :])

            for rep in range(reps):
              # feat readout quarter q: k-tiles [q*KH/2, ...) for both classes
              def emit_feat_readout(q):
                  for c in range(C if use_readout else 0):
                      lo = q * (KT0 // NLAY)
                      hi = lo + KT0 // NLAY
                      for k0 in range(lo, hi, CH):
                          cc = min(CH, hi - k0)
                          wt = stream.tile([128, CH, NPC], dt.bfloat16, tag="wro",
                                           bufs=2, name="wt")
                          nc.sync.dma_start(wt[:, :cc, :], wp0[c, :, k0:k0 + cc, :])
                          for kk in range(cc):
                              k = k0 + kk
                              scr = stream.tile([128, NPC], dt.float32, tag="scr",
                                                name="scr")
                              nc.vector.scalar_tensor_tensor(
                                  out=scr[:], in0=featT_sb[:, k, :], scalar=1.0,
                                  in1=wt[:, kk, :], op0=ALU.mult, op1=ALU.mult,
                                  accum_out=racc[:, c * NSLOT + k: c * NSLOT + k + 1])

              hT_sb = None
              for lay in range(NLAY):
                  K = N if lay == 0 else H
                  KT = K // 128

                  # ---- prefetch Wb for this layer (lands during AG wait) ----
                  wb_sb = sb.tile([128, HT, H], dt.bfloat16, tag="wb")
                  nc.sync.dma_start(wb_sb[:], wb[lay])

                  # ---- Z_c = h_c @ Wa  (node-major out [NPC, H]) ----
                  psZ = [psum.tile([128, H], dt.float32, tag="acc", name=f"psZ{m}") for m in range(MT)]
                  wsrc = w0a if lay == 0 else wra[lay - 1]
                  for k0 in range(0, KT, CH):
                      cc = min(CH, KT - k0)
                      if lay == 0 and rep == 0:
                          load_featT_chunk(k0)
                      wa_t = stream.tile([128, CH, H], dt.bfloat16, tag="wa", bufs=2)
                      nc.sync.dma_start(wa_t[:, :cc, :], wsrc[:, k0:k0 + cc, :])
                      for kk in range(cc):
                          k = k0 + kk
                          lhs_tile = featT_sb[:, k, :] if lay == 0 else hT_sb[:, k, :]
                          for m in range(MT):
                              nc.tensor.matmul(
                                  psZ[m][:], lhsT=lhs_tile[:, m * 128:(m + 1) * 128],
                                  rhs=wa_t[:, kk, :], start=(k == 0), stop=(k == KT - 1))

                  if lay == 0 and rep == 0:
                      # constants land well before first use (diag MMs, stage1)
                      nc.sync.dma_start(aff_sb[:], aff[:])
                      nc.sync.dma_start(identl_sb[:], identl[:])

                  # ---- node-major Z copy (scalar engine; bf16 cast) ----
                  zcat = sb.tile([128, MT, H], dt.bfloat16, tag="zcat")
                  for m in range(MT):
                      nc.scalar.activation(zcat[:, m, :], psZ[m][:], AF.Copy)

                  # ---- AllGather Z in two node-halves ----
                  zsrcs = []
                  for half in range(2):
                      zin_h = dram.tile([NPC // 2, H], dt.bfloat16, tag=f"zin{half}")
                      nc.sync.dma_start(
                          zin_h.rearrange("(m p) h -> p m h", p=128),
                          zcat[:, 2 * half:2 * half + 2, :])
                      if use_coll:
                          zfull_h = dram.tile([N // 2, H], dt.bfloat16,
                                              tag=f"zfull{half}", addr_space="Shared")
                          nc.gpsimd.collective_compute(
                              "AllGather", ALU.bypass, replica_groups=rg,
                              ins=[zin_h.opt()], outs=[zfull_h.opt()])
                          zsrcs.append(zfull_h)
                      else:
                          zsrcs.append(zdummy)

                  # ---- XT_c = Zfull.T @ A_T_c + (1+eps) Z_c^T -> [H, NPC] ----
                  psX = [psum.tile([128, NPC], dt.float32, tag="acc", name=f"psX{m}") for m in range(HT)]
                  # (1+eps) Z_c^T from local node-major Z; runs during the AG.
                  # mn=0's start=True marks the whole PSUM bank pending-zero;
                  # mn=1..3 overwrite their (still-pending) column blocks.
                  for mn in range(MT):
                      for m in range(HT):
                          nc.tensor.matmul(
                              psX[m][:, mn * 128:(mn + 1) * 128],
                              lhsT=zcat[:, mn, m * 128:(m + 1) * 128],
                              rhs=identl_sb[:, lay, :],
                              start=(mn == 0), stop=False)

                  # prefetch this layer's readout weights during the AG wait
                  wrr_tiles = []
                  for c in range(C if use_readout else 0):
                      wt = stream.tile([128, HT, NPC], dt.bfloat16, tag="wrr", bufs=2)
                      nc.sync.dma_start(wt[:], wpr[lay, c])
                      wrr_tiles.append(wt)

                  zf_sb = sb.tile([128, KT0, H], dt.bfloat16, tag="zf", bufs=1)
                  for half in range(2):
                      if lay == 0 and rep == 0:
                          for k0 in range(half * KH, (half + 1) * KH, CH):
                              load_at_chunk(k0)
                      # small first chunk so the X matmuls start sooner
                      for t0, cc in ((0, 4), (4, 8), (12, 4)):
                          nc.sync.dma_start(
                              zf_sb[:, half * KH + t0:half * KH + t0 + cc, :],
                              zsrcs[half][t0 * 128:(t0 + cc) * 128, :].rearrange("(t p) h -> p t h", p=128))
                      for k in range(half * KH, (half + 1) * KH):
                          for m in range(HT):
                              nc.tensor.matmul(
                                  psX[m][:], lhsT=zf_sb[:, k, m * 128:(m + 1) * 128],
                                  rhs=at_sb[:, k, :], start=False, stop=(k == KT0 - 1))

                  # ---- stage 1: relu(X * s1 + t1), cast bf16 ----
                  xt_sb = sb.tile([128, HT, NPC], dt.bfloat16, tag="xt")
                  for m in range(HT):
                      nc.scalar.activation(
                          xt_sb[:, m, :], psX[m][:], AF.Relu,
                          bias=aff_sb[:, aff_col(lay, 1, m):aff_col(lay, 1, m) + 1],
                          scale=aff_sb[:, aff_col(lay, 0, m):aff_col(lay, 0, m) + 1])

                  # ---- YT_c = Wb.T @ XT_c -> [H, NPC] ----
                  # m-outer: psY[m] completes early so stage2/3 + next-layer Z
                  # pipeline behind it (also eases PSUM slot pressure)
                  psY = [psum.tile([128, NPC], dt.float32, tag="acc", name=f"psY{m}") for m in range(HT)]
                  for m in range(HT):
                      for k in range(HT):
                          nc.tensor.matmul(
                              psY[m][:], lhsT=wb_sb[:, k, m * 128:(m + 1) * 128],
                              rhs=xt_sb[:, k, :], start=(k == 0), stop=(k == HT - 1))

                  # ---- stages 2+3: two fused affine+relu, cast bf16 ----
                  hT_sb = sb.tile([128, HT, NPC], dt.bfloat16, tag="hT")
                  for m in range(HT):
                      tmp = sb.tile([128, NPC], dt.float32, tag="tmp")
                      nc.scalar.activation(
                          tmp[:], psY[m][:], AF.Relu,
                          bias=aff_sb[:, aff_col(lay, 3, m):aff_col(lay, 3, m) + 1],
                          scale=aff_sb[:, aff_col(lay, 2, m):aff_col(lay, 2, m) + 1])
                      nc.scalar.activation(
                          hT_sb[:, m, :], tmp[:], AF.Relu,
                          bias=aff_sb[:, aff_col(lay, 5, m):aff_col(lay, 5, m) + 1],
                          scale=aff_sb[:, aff_col(lay, 4, m):aff_col(lay, 4, m) + 1])

                  # ---- readout for this hidden rep ----
                  for c in range(C if use_readout else 0):
                      wt = wrr_tiles[c]
                      for m in range(HT):
                          scr = stream.tile([128, NPC], dt.float32, tag="scr")
                          slot = c * NSLOT + KT0 + lay * HT + m
                          nc.vector.scalar_tensor_tensor(
                              out=scr[:], in0=hT_sb[:, m, :], scalar=1.0,
                              in1=wt[:, m, :], op0=ALU.mult, op1=ALU.mult,
                              accum_out=racc[:, slot:slot + 1])

                  # feat readout quarters (fill AG bubbles); quarter 3 is
                  # emitted with layer 2 so the last layer has no DVE tail
                  for q in {0: [0], 1: [1], 2: [2, 3], 3: []}[lay]:
                      emit_feat_readout(q)

                  # ---- HAM warm-keepers: lowest-priority junk matmuls.
                  # Ready as soon as this layer's zcat lands (= AG start), so
                  # they fill the AllGather PE-idle window and keep the PE
                  # clock-gate at full rate; real work preempts them.
                  wps = warm.tile([128, NPC], dt.float32, tag="warm")
                  for i in range(56):
                      nc.tensor.matmul(
                          wps[:], lhsT=zcat[:, i % MT, 0:128],
                          rhs=at_sb[:, i % KT0, :], start=True, stop=True)

              # ---- finish readout: free-dim reduce; partition sum on host ----
              r2 = sb.tile([128, C], dt.float32, tag="r2")
              for c in range(C):
                  nc.vector.tensor_reduce(
                      r2[:, c:c + 1], racc[:, c * NSLOT:(c + 1) * NSLOT],
                      axis=mybir.AxisListType.X, op=ALU.add)
              nc.sync.dma_start(score_part[:], r2[:])

    nc.compile()
    return nc


def prep_inputs(inputs, N=N_FULL, H=H_FULL, ncores=NCORES, nlay=NLAY):
    """Host-side re-layout of the full inputs into per-core input maps.

    Returns (in_maps, bias_tot)."""
    inp = {k: np.asarray(v) for k, v in inputs.items()}
    NPC = N // ncores
    HT = H // 128
    f32 = np.float32

    feat = inp["feat"].astype(f32)
    src = inp["edge_src"].astype(np.int64)
    dst = inp["edge_dst"].astype(np.int64)

    A_T = np.zeros((N, N), f32)
    np.add.at(A_T, (src, dst), 1.0)
    eps_list = [float(inp["eps0"])] + [float(x) for x in inp["epsR"]]

    # Row order matching the two-half AllGather: half g of every core's
    # node block, cores in rank order.
    half = NPC // 2
    r = np.arange(N // 2)
    perm = np.concatenate([(r // half) * NPC + (r % half),
                           (r // half) * NPC + half + (r % half)])
    at_perm = np.ascontiguousarray(A_T[perm]).astype(bf16)

    def pack(x):
        # [T*128, F] row-major -> partition-major [128, T, F]
        T = x.shape[0] // 128
        return np.ascontiguousarray(
            x.reshape(T, 128, x.shape[1]).transpose(1, 0, 2))

    identl = np.zeros((128, nlay, 128), f32)
    for i in range(nlay):
        identl[:, i, :] = (1.0 + eps_list[i]) * np.eye(128, dtype=f32)
    identl = identl.astype(bf16)

    featT = np.ascontiguousarray(feat.T).astype(bf16)
    w0a = pack(inp["W0a"].astype(f32).astype(bf16))
    wra = np.stack([pack(w) for w in inp["WRa"].astype(f32).astype(bf16)])
    wb = np.stack([pack(w) for w in
                   np.concatenate([inp["W0b"][None], inp["WRb"]],
                                  axis=0).astype(f32).astype(bf16)])

    ba = [inp["b0a"]] + [inp["bRa"][i] for i in range(nlay - 1)]
    bb = [inp["b0b"]] + [inp["bRb"][i] for i in range(nlay - 1)]

    def fold(nm, i):
        idx = (lambda x: x) if i == 0 else (lambda x: x[i - 1])
        g, b_, m, v = [idx(inp[nm + s]) for s in ("_g", "_b", "_m", "_v")]
        s = (g / np.sqrt(v + 1e-5)).astype(f32)
        return s, b_, m

    # aff[p, lay*6*HT + stage*HT + m] with stages (s1,t1,s2,t2,s3,t3)
    aff = np.zeros((128, nlay * 6 * HT), f32)
    for i in range(nlay):
        nms = ("bn0a", "bnA0", "bnO0") if i == 0 else ("bnRa", "bnAR", "bnOR")
        s, b_, m = fold(nms[0], i)
        p1s, p1t = s, ((ba[i] - m) * s + b_).astype(f32)
        s, b_, m = fold(nms[1], i)
        p2s, p2t = s, ((bb[i] - m) * s + b_).astype(f32)
        s, b_, m = fold(nms[2], i)
        p3s, p3t = s, (b_ - m * s).astype(f32)
        for mi in range(HT):
            sl = slice(mi * 128, (mi + 1) * 128)
            for j, vec in enumerate((p1s, p1t, p2s, p2t, p3s, p3t)):
                aff[:, i * 6 * HT + j * HT + mi] = vec[sl]

    wp0r = np.ascontiguousarray(
        inp["Wp0"].astype(f32).reshape(N, N, C).transpose(2, 1, 0)).astype(bf16)
    wprr = np.ascontiguousarray(
        inp["WpR"].astype(f32).reshape(nlay, N, H, C).transpose(0, 3, 2, 1)).astype(bf16)
    bias_tot = (inp["bp0"] + inp["bpR"].sum(axis=0)).astype(f32).reshape(1, 2)

    in_maps = []
    for cix in range(ncores):
        sl = slice(cix * NPC, (cix + 1) * NPC)
        in_maps.append({
            "featT": pack(featT[:, sl]),
            "w0a": w0a,
            "wra": wra,
            "wb": wb,
            "at": pack(at_perm[:, sl]),
            "identl": identl,
            "wp0": np.stack([pack(wp0r[c, :, sl]) for c in range(2)]),
            "wpr": np.stack([np.stack([pack(wprr[l, c, :, sl]) for c in range(2)])
                             for l in range(nlay)]),
            "aff": aff,
        })
    return in_maps, bias_tot


_CACHE = {}


def _get_program():
    if "nc" not in _CACHE:
        _CACHE["nc"] = build_program()
    return _CACHE["nc"]


def kernel(**inputs):
    nc = _get_program()
    in_maps, bias_tot = prep_inputs(inputs)
    res = run_bass_kernel_spmd(nc, in_maps, list(range(NCORES)))
    acc = np.zeros((1, C), np.float64)
    for cix in range(NCORES):
        acc += res.results[cix]["score_part"].astype(np.float64).sum(axis=0)
    return (acc + bias_tot).astype(np.float32)
